# revision 1
# baseline (speedup 1.0000x reference)
"""GAT layer kernel for Trainium2, data-parallel over 8 NeuronCores.

Problem (per graph): X [1024, 128] f32, W [64, 128], a [1, 128]
  h = X @ W.T                       [1024, 64]
  s_src = h @ a[:64], s_dst = h @ a[64:]
  e[i,j] = leaky_relu(s_src[i] + s_dst[j], 0.01)
  att = softmax_j(e); out = att @ h  [1024, 64]

32 graphs total -> 4 per core across 8 cores (inputs W/a replicated).

Per-core kernel strategy (all tiles 128-partition):
  - Attention is built directly in TRANSPOSED tile layout
    PT[j, i] = exp(lrelu(s_src[i] + s_dst[j])), tiles [128 j x 1024 i],
    which is exactly the lhsT the TensorE needs for att @ h - no big
    transposes.  exp(lrelu(x)) = max(exp(x), exp(x/100)) since exp is
    monotonic, so no Lrelu activation needed:
        PT = max(A[i]*B[j], C[i]*D[j])
        A = exp(s_src), B = exp(s_dst), C = exp(s_src/100), D = exp(s_dst/100)
  - s_dst columns fall out of the h matmul as an extra rhs column.
  - s_src arrives already replicated across partitions: the score matmul
    uses a column-replicated weight vector as the stationary operand
    (wsrc_rep[f, m] = w_src[f] for all m), so PSUM gets
    srep[m, i] = s_src[i] on every partition m directly.  One exp per
    graph turns that into replicated SBUF tiles A_rep / C_rep (bf16);
    per j-tile the pass is just tensor_scalar + scalar_tensor_tensor on
    the vector engine.
  - A ones column is appended to h so the accumulation matmul
    PT.T @ [h | 1] produces both h' and the softmax normalizer Z in PSUM;
    the epilogue multiplies by 1/Z per partition.
"""

import os
import sys

if "/opt/trn_rl_repo" not in sys.path:
    sys.path.insert(0, "/opt/trn_rl_repo")

from contextlib import ExitStack

import numpy as np

import concourse.bass as bass
import concourse.mybir as mybir
import concourse.tile as tile
from concourse import bacc
from concourse.bass_utils import run_bass_kernel_spmd
from concourse.masks import make_identity

# ---- hardcoded problem shapes -------------------------------------------
N_TOTAL = 32          # graphs
N_CORES = 8
N_PER = N_TOTAL // N_CORES   # 4 graphs per core
V = 1024              # nodes per graph
F = 128               # input features
H = 64                # hidden features
NT = V // 128         # 8 tiles of 128 nodes
SLOPE = 0.01          # leaky_relu negative slope
P_MODE = os.environ.get("GAT_P_MODE", "ts_tt")  # "stt" | "ts_tt"
SKIP = os.environ.get("GAT_SKIP", "")          # "accmm" | "pbuild" | "exps" | "loopa"
COPY_ENG = os.environ.get("GAT_COPY_ENG", "act")  # "act" | "dve"
MAXPOOL_N = int(os.environ.get("GAT_MAXPOOL", "0"))  # tiles/graph whose max runs on gpsimd
T1ACT_N = int(os.environ.get("GAT_T1ACT", "0"))  # tiles/graph whose t1 comes from ScalarE exp

FP32 = mybir.dt.float32
BF16 = mybir.dt.bfloat16
AF = mybir.ActivationFunctionType
OP = mybir.AluOpType


def build_gat_program(reps: int = 1):
    """Build the per-core Bass program (same program on all 8 cores).

    reps > 1 repeats the whole per-core pipeline (for device-time
    measurement by differencing); all reps write the same outputs.
    """
    nc = bacc.Bacc("TRN2", target_bir_lowering=False, debug=False)

    feat_d = nc.dram_tensor("features", [N_PER, V, F], FP32, kind="ExternalInput")
    w_d = nc.dram_tensor("W", [H, F], FP32, kind="ExternalInput")
    a_d = nc.dram_tensor("a", [1, 2 * H], FP32, kind="ExternalInput")
    out_d = nc.dram_tensor("out", [N_PER, V, H], FP32, kind="ExternalOutput")

    feat = feat_d.ap()
    out = out_d.ap()

    with tile.TileContext(nc) as tc, ExitStack() as ctx:
        # ---- pools -------------------------------------------------------
        consts = ctx.enter_context(tc.tile_pool(name="consts", bufs=1))
        xpool = ctx.enter_context(tc.tile_pool(name="x", bufs=4))
        xtpool = ctx.enter_context(tc.tile_pool(name="xt", bufs=3))
        augpool = ctx.enter_context(tc.tile_pool(name="aug", bufs=2 * NT))
        sdpool = ctx.enter_context(tc.tile_pool(name="sd", bufs=2))
        reppool = ctx.enter_context(tc.tile_pool(name="rep", bufs=2))
        t2pool = ctx.enter_context(tc.tile_pool(name="t2", bufs=3))
        ppool = ctx.enter_context(tc.tile_pool(name="p", bufs=2 * NT))
        rzpool = ctx.enter_context(tc.tile_pool(name="rz", bufs=2))
        opool = ctx.enter_context(tc.tile_pool(name="o", bufs=2))

        # PSUM bank budget (8 total): ps_t=2, ps_h=2, ps_srep=2, ps_out=2
        ps_t = ctx.enter_context(tc.tile_pool(name="ps_t", bufs=2, space="PSUM"))
        ps_h = ctx.enter_context(tc.tile_pool(name="ps_h", bufs=2, space="PSUM"))
        ps_srep = ctx.enter_context(tc.tile_pool(name="ps_srep", bufs=1, space="PSUM"))
        ps_out = ctx.enter_context(tc.tile_pool(name="ps_out", bufs=2, space="PSUM"))

        # ---- constants / weight prep ------------------------------------
        ident = consts.tile([128, 128], FP32)
        make_identity(nc, ident[:])

        a_sb = consts.tile([1, 2 * H], FP32)
        nc.sync.dma_start(a_sb[:], a_d.ap()[:])
        w_sb = consts.tile([H, F], FP32)
        nc.sync.dma_start(w_sb[:], w_d.ap()[:])
        wb = consts.tile([H, F], BF16)
        nc.vector.tensor_copy(wb[:], w_sb[:])

        # a halves -> bf16 columns [H, 2] (via PE transpose of the row)
        asrc_ps = ps_t.tile([H, 1], FP32, tag="xt")
        nc.tensor.transpose(asrc_ps[:], a_sb[0:1, 0:H], ident[0:1, 0:1])
        adst_ps = ps_t.tile([H, 1], FP32, tag="xt")
        nc.tensor.transpose(adst_ps[:], a_sb[0:1, H : 2 * H], ident[0:1, 0:1])
        a2 = consts.tile([H, 2], FP32)
        nc.vector.tensor_copy(a2[:, 0:1], asrc_ps[:])
        nc.vector.tensor_copy(a2[:, 1:2], adst_ps[:])

        # w_src/w_dst = W.T @ a_halves : [F, 2] (f32 psum, fp32 one-time matmul)
        wcols_ps = ps_t.tile([F, 2], FP32, tag="xt")
        nc.tensor.matmul(wcols_ps[:], lhsT=w_sb[:], rhs=a2[:], start=True, stop=True)
        # column-replicated w_src: wsrc_rep[f, m] = w_src[f] for all m
        wsrc_rep = consts.tile([F, 128], BF16)
        nc.scalar.copy(wsrc_rep[:], wcols_ps[:, 0:1].broadcast_to((F, 128)))

        # rhs_w = [w_dst | W.T] : [F, 1+H] bf16
        wt_ps = ps_t.tile([F, H], FP32, tag="xt")
        nc.tensor.transpose(wt_ps[:], w_sb[:], ident[0:H, 0:H])
        rhs_w = consts.tile([F, 1 + H], BF16)
        nc.vector.tensor_copy(rhs_w[:, 0:1], wcols_ps[:, 1:2])
        nc.vector.tensor_copy(rhs_w[:, 1 : 1 + H], wt_ps[:])

        # ---- per-graph pipeline -----------------------------------------
        for rep in range(reps):
          for g in range(N_PER):
            # whole-graph X load: [1024, 128] as one DMA of [128, 8*128]
            fg = feat[g].rearrange("(q p) c -> p q c", q=8, p=128)
            xq = xpool.tile([128, NT * F], FP32, name=f"xq_{rep}_{g}", tag="xq")
            nc.sync.dma_start(xq[:].rearrange("p (q c) -> p q c", q=8), fg)

            # -- loop A: transpose tiles, h-matmul, replicated s_src ------
            srep_ps = ps_srep.tile([128, V], FP32)
            sdst_g = sdpool.tile([128, NT], FP32)
            augs = []
            for jt in range(NT):
                x_t = xq[:, jt * F : (jt + 1) * F]

                xt_ps = ps_t.tile([128, 128], FP32, tag="xt")
                nc.tensor.transpose(xt_ps[:], x_t, ident[:])
                xtb = xtpool.tile([128, 128], BF16)
                (nc.scalar.copy if COPY_ENG == "act" else nc.vector.tensor_copy)(
                    xtb[:], xt_ps[:])

                # [s_dst | h] for this node tile
                h_ps = ps_h.tile([128, 1 + H], FP32)
                nc.tensor.matmul(h_ps[:], lhsT=xtb[:], rhs=rhs_w[:], start=True, stop=True)

                aug = augpool.tile([128, 2 + H], BF16)
                nc.gpsimd.memset(aug[:, 1 + H : 2 + H], 1.0)
                (nc.scalar.copy if COPY_ENG == "act" else nc.vector.tensor_copy)(
                    aug[:, 0 : 1 + H], h_ps[:])
                augs.append(aug)

                nc.scalar.copy(sdst_g[:, jt : jt + 1], h_ps[:, 0:1])

                # replicated s_src segment: [128, 128], every row = s_src
                nc.tensor.matmul(
                    srep_ps[:, jt * 128 : (jt + 1) * 128],
                    lhsT=wsrc_rep[:],
                    rhs=xtb[:],
                    start=True,
                    stop=True,
                )

            # -- A2: exp everything ---------------------------------------
            a_rep = reppool.tile([128, V], BF16, tag="a_rep")
            c_rep = reppool.tile([128, V], BF16, tag="c_rep")
            if SKIP == "exps":
                nc.scalar.activation(a_rep[:, 0:64], srep_ps[:, 0:64], AF.Exp)
                nc.scalar.activation(c_rep[:, 0:64], srep_ps[:, 0:64], AF.Exp, scale=SLOPE)
            else:
                # halves: each can start once the first/last 4 srep matmuls land
                for hv in range(2):
                    sl = slice(hv * 512, (hv + 1) * 512)
                    nc.scalar.activation(a_rep[:, sl], srep_ps[:, sl], AF.Exp)
                    nc.scalar.activation(c_rep[:, sl], srep_ps[:, sl], AF.Exp, scale=SLOPE)

            b_g = sdpool.tile([128, NT], FP32, tag="b_g")
            nc.scalar.activation(b_g[:], sdst_g[:], AF.Exp)
            d_g = sdpool.tile([128, NT], FP32, tag="d_g")
            nc.scalar.activation(d_g[:], sdst_g[:], AF.Exp, scale=SLOPE)

            # -- loop B: attention tiles, then accumulation matmuls -------
            # (each PSUM accumulation group runs start->stop back-to-back so
            #  no two pending groups share a bank)
            po = [ps_out.tile([128, 4 * (H + 1)], FP32, name=f"po_{rep}_{g}_{i}", tag="po")
                  for i in range(2)]
            p_ts = []
            _pv = 64 if SKIP == "pbuild" else V
            for jt in range(NT):
                t2 = t2pool.tile([128, V], BF16)
                nc.vector.tensor_scalar(
                    t2[:, 0:_pv], c_rep[:, 0:_pv], d_g[:, jt : jt + 1], None, OP.mult
                )
                p_t = ppool.tile([128, V], BF16)
                if P_MODE == "stt":
                    nc.vector.scalar_tensor_tensor(
                        p_t[:, 0:_pv], in0=a_rep[:, 0:_pv], scalar=b_g[:, jt : jt + 1],
                        in1=t2[:, 0:_pv], op0=OP.mult, op1=OP.max,
                    )
                else:  # ts + tensor_tensor max
                    t1 = t2pool.tile([128, V], BF16, tag="t1")
                    if jt < T1ACT_N:
                        # t1 = exp(s_src + d_j) straight from PSUM on ScalarE
                        nc.scalar.activation(
                            t1[:, 0:_pv], srep_ps[:, 0:_pv], AF.Exp,
                            bias=sdst_g[:, jt : jt + 1],
                        )
                    else:
                        nc.vector.tensor_scalar(
                            t1[:, 0:_pv], a_rep[:, 0:_pv], b_g[:, jt : jt + 1], None, OP.mult
                        )
                    max_eng = nc.gpsimd if jt < MAXPOOL_N else nc.vector
                    max_eng.tensor_tensor(
                        p_t[:, 0:_pv], t1[:, 0:_pv], t2[:, 0:_pv], OP.max
                    )
                p_ts.append(p_t)
            for it in range(NT if SKIP != "accmm" else 0):
                t, r = it // 4, it % 4
                for jt in range(NT):
                    nc.tensor.matmul(
                        po[t][:, r * (H + 1) : (r + 1) * (H + 1)],
                        lhsT=p_ts[jt][:, it * 128 : (it + 1) * 128],
                        rhs=augs[jt][:, 1 : 2 + H],
                        start=(jt == 0),
                        stop=(jt == NT - 1),
                    )

            # -- loop C: normalize + single batched store -----------------
            o_g = opool.tile([128, NT * H], FP32)
            for it in range(NT if SKIP != "accmm" else 0):
                t, r = it // 4, it % 4
                base = r * (H + 1)
                rz = rzpool.tile([128, 1], FP32)
                nc.vector.reciprocal(rz[:], po[t][:, base + H : base + H + 1])
                nc.vector.tensor_scalar(
                    o_g[:, it * H : (it + 1) * H],
                    po[t][:, base : base + H], rz[:], None, OP.mult,
                )
            if SKIP != "accmm":
                og_dst = out[g].rearrange("(it p) c -> p it c", it=NT, p=128)
                nc.sync.dma_start(og_dst, o_g[:].rearrange("p (it c) -> p it c", it=NT))

    nc.compile()
    return nc


_NC_CACHE = None


def _get_program():
    global _NC_CACHE
    if _NC_CACHE is None:
        _NC_CACHE = build_gat_program()
    return _NC_CACHE


def kernel(features: np.ndarray, W: np.ndarray, a: np.ndarray) -> np.ndarray:
    """Full-input entry point: features [32, 1024, 128], W [64, 128], a [1, 128]."""
    assert features.shape == (N_TOTAL, V, F)
    nc = _get_program()

    features = np.ascontiguousarray(features, dtype=np.float32)
    W = np.ascontiguousarray(W, dtype=np.float32)
    a = np.ascontiguousarray(a, dtype=np.float32)

    in_maps = [
        {
            "features": features[c * N_PER : (c + 1) * N_PER],
            "W": W,
            "a": a,
        }
        for c in range(N_CORES)
    ]
    res = run_bass_kernel_spmd(nc, in_maps, core_ids=list(range(N_CORES)))
    outs = [res.results[c]["out"] for c in range(N_CORES)]
    return np.concatenate(outs, axis=0)


if __name__ == "__main__":
    prog = build_gat_program()
    print("program built ok")

